# revision 10
# baseline (speedup 1.0000x reference)
"""ArcFace head kernel for 8 Trainium2 NeuronCores.

out[n, c] = S * cos(n, c)                    for c != labels[n]
out[n, y] = S * (cos_y*cos(M) - sqrt(1-cos_y^2)*sin(M))   (y = labels[n])
where cos = l1norm(emb) @ l1norm(weight).T

Sharding: weight rows (classes) split across 8 cores (12544 classes each,
zero-padded from 100000 to 100352). Each core computes its [2048, 12544]
logit slab; the host concatenates the slabs and trims the padding.

Per-core device pipeline:
  - emb loaded naturally, L1-normalized (fused |x| reduce), scaled by
    S/||x||_1, cast to bf16, transposed on the PE into x^T k-chunks.
  - weight panels (512 classes) loaded naturally, L1-normalized, cast to
    bf16, PE-transposed into w^T panels.
  - bf16 matmuls accumulate D=512 (4 k-chunks) into PSUM; ScalarE/VectorE
    drain PSUM into an SBUF staging panel; one 4MB DMA per panel writes the
    output slab.
  - margin fixup: indirect-DMA gather of out[n, labels[n]] (clamped local
    column), margin computed on-device (identity cos(th+M) =
    c*cosM - sqrt(1-c^2)*sinM, so no arccos needed), indirect-DMA scatter
    back with out-of-range rows dropped via the bounds check.
"""

import math
import os
import sys

import numpy as np

for _p in ("/opt/trn_rl_repo", "/opt/pypackages"):
    if os.path.isdir(_p) and _p not in sys.path:
        sys.path.append(_p)

import concourse.bass as bass
import concourse.tile as tile
from concourse import bacc, mybir
from concourse.bass import IndirectOffsetOnAxis
from concourse.bass_utils import run_bass_kernel_spmd
from concourse.masks import make_identity
from bass_rust import add_dep_helper

P = 128
S = 30.0
MARGIN = 0.5
EPS_NORM = 1e-12
EPS_CLIP = 1e-7

N_CORES = 8
N_FULL = 2048
D_FULL = 512
C_FULL = 100000
CS = 12544          # classes per core (98 * 128); 8*CS = 100352 >= C_FULL
OOB_SENTINEL = 1 << 28  # scatter index for rows whose label is not local

LAST_EXEC_NS = None
LAST_RESULTS = None

f32 = mybir.dt.float32
bf16 = mybir.dt.bfloat16
i32 = mybir.dt.int32
ALU = mybir.AluOpType
AX = mybir.AxisListType


def build_arcface(n=N_FULL, d=D_FULL, cs=CS, panel_w=512):
    """Build the single-core Bass graph (SPMD: same graph on all 8 cores)."""
    assert n % P == 0 and d % P == 0 and cs % P == 0
    nt = n // P          # row tiles
    kc = d // P          # contraction chunks
    panels = []
    c = cs
    while c > 0:
        w = min(panel_w, c)
        assert w % P == 0
        panels.append(w)
        c -= w

    # Bacc (not raw Bass): its compile() pass splits multi-sem sync waits to
    # the 1-wait-per-instruction limit of this toolchain's walrus codegen.
    nc = bacc.Bacc()
    emb_h = nc.declare_dram_parameter("emb", [n, d], f32, isOutput=False)
    w_h = nc.declare_dram_parameter("weight", [cs, d], f32, isOutput=False)
    gg_h = nc.declare_dram_parameter("gidxg", [P, nt], i32, isOutput=False)
    gs_h = nc.declare_dram_parameter("gidxs", [P, nt], i32, isOutput=False)
    out_h = nc.declare_dram_parameter("out", [n, cs], f32, isOutput=True)

    with tile.TileContext(nc) as tc:
        with (
            tc.tile_pool(name="consts", bufs=1) as consts,
            tc.tile_pool(name="xnat", bufs=3) as xnat_p,
            tc.tile_pool(name="xs", bufs=3) as xs_p,
            tc.tile_pool(name="stats", bufs=24) as stats,
            tc.tile_pool(name="wn", bufs=2) as wn_p,
            tc.tile_pool(name="ws", bufs=3) as ws_p,
            tc.tile_pool(name="wT", bufs=2) as wT_p,
            tc.tile_pool(name="stage", bufs=2) as stage_p,
            tc.tile_pool(name="fix", bufs=16) as fix_p,
            tc.tile_pool(name="pmm", bufs=4, space="PSUM") as pmm_p,
            tc.tile_pool(name="ptr", bufs=2, space="PSUM") as ptr_p,
        ):
            ident = consts.tile([P, P], bf16)
            make_identity(nc, ident)
            gg_sb = consts.tile([P, nt], i32)
            gs_sb = consts.tile([P, nt], i32)
            nc.sync.dma_start(out=gg_sb, in_=gg_h[:, :])
            nc.sync.dma_start(out=gs_sb, in_=gs_h[:, :])

            # x^T, kept resident: [P, kc, n] bf16, pre-scaled by S/||x||_1
            xT = consts.tile([P, kc, n], bf16)
            for t in range(nt):
                xn = xnat_p.tile([P, d], f32)
                nc.sync.dma_start(out=xn, in_=emb_h[P * t : P * (t + 1), :])
                xnorm = stats.tile([P, 1], f32, tag="xnorm")
                nc.vector.tensor_reduce(
                    out=xnorm, in_=xn, axis=AX.X, op=ALU.add,
                    apply_absolute_value=True,
                )
                xnorm2 = stats.tile([P, 1], f32, tag="xnorm2")
                nc.vector.tensor_scalar(
                    out=xnorm2, in0=xnorm, scalar1=EPS_NORM, scalar2=None,
                    op0=ALU.max,
                )
                xr = stats.tile([P, 1], f32, tag="xr")
                nc.vector.reciprocal(out=xr, in_=xnorm2)
                xrs = stats.tile([P, 1], f32, tag="xrs")
                nc.vector.tensor_scalar(
                    out=xrs, in0=xr, scalar1=S, scalar2=None, op0=ALU.mult,
                )
                xs = xs_p.tile([P, d], bf16)
                nc.vector.tensor_scalar(
                    out=xs, in0=xn, scalar1=xrs, scalar2=None, op0=ALU.mult,
                )
                px = ptr_p.tile([P, kc, P], bf16, tag="ptr")
                for k in range(kc):
                    nc.tensor.transpose(
                        out=px[:, k, :], in_=xs[:, P * k : P * (k + 1)],
                        identity=ident,
                    )
                nc.vector.tensor_copy(out=xT[:, :, P * t : P * (t + 1)], in_=px)

            out_view = out_h[:, :].rearrange("(t p) c -> p t c", p=P)
            out_dmas = []
            cstart = 0
            for pw in panels:
                jw = pw // P
                wn = wn_p.tile([P, jw, d], f32, tag="wn")
                nc.sync.dma_start(
                    out=wn,
                    in_=w_h[cstart : cstart + pw, :].rearrange(
                        "(j p) d -> p j d", p=P
                    ),
                )
                wT = wT_p.tile([P, kc, pw], bf16, tag="wT")
                for j in range(jw):
                    wnorm = stats.tile([P, 1], f32, tag="wnorm")
                    nc.vector.tensor_reduce(
                        out=wnorm, in_=wn[:, j, :], axis=AX.X, op=ALU.add,
                        apply_absolute_value=True,
                    )
                    wnorm2 = stats.tile([P, 1], f32, tag="wnorm2")
                    nc.vector.tensor_scalar(
                        out=wnorm2, in0=wnorm, scalar1=EPS_NORM, scalar2=None,
                        op0=ALU.max,
                    )
                    wr = stats.tile([P, 1], f32, tag="wr")
                    nc.vector.reciprocal(out=wr, in_=wnorm2)
                    ws = ws_p.tile([P, d], bf16)
                    nc.vector.tensor_scalar(
                        out=ws, in0=wn[:, j, :], scalar1=wr, scalar2=None,
                        op0=ALU.mult,
                    )
                    ptr = ptr_p.tile([P, kc, P], bf16, tag="ptr")
                    for k in range(kc):
                        nc.tensor.transpose(
                            out=ptr[:, k, :], in_=ws[:, P * k : P * (k + 1)],
                            identity=ident,
                        )
                    nc.vector.tensor_copy(
                        out=wT[:, :, P * j : P * (j + 1)], in_=ptr
                    )

                stage = stage_p.tile([P, nt, pw], f32, tag="stage")
                for t in range(nt):
                    pmm = pmm_p.tile([P, pw], f32, tag="pmm")
                    for k in range(kc):
                        nc.tensor.matmul(
                            out=pmm,
                            lhsT=xT[:, k, P * t : P * (t + 1)],
                            rhs=wT[:, k, :],
                            start=(k == 0),
                            stop=(k == kc - 1),
                        )
                    if t % 4 != 3:
                        nc.scalar.copy(out=stage[:, t, :], in_=pmm)
                    else:
                        nc.vector.tensor_copy(out=stage[:, t, :], in_=pmm)
                dd = nc.sync.dma_start(
                    out=out_view[:, :, cstart : cstart + pw], in_=stage
                )
                out_dmas.append(dd.ins)
                cstart += pw

            # ---- margin fixup --------------------------------------------
            out_flat = bass.AP(
                tensor=out_h[:, :].tensor, offset=0, ap=[[1, n * cs], [1, 1]]
            )
            # One barrier nop absorbs the waits on all panel out-DMAs, so the
            # gather DMA instructions themselves need no sync-wait commands
            # (hardware limits waits per DMA instruction).
            barrier = nc.gpsimd.nop(nofuse=True, hint="fixup_barrier")
            for dins in out_dmas:
                add_dep_helper(barrier.ins, dins, True, "fixup waits for slab")
            gat = fix_p.tile([P, nt], f32, tag="gat")
            gather_insts = []
            for t in range(nt):
                gi = nc.gpsimd.indirect_dma_start(
                    out=gat[:, t : t + 1],
                    out_offset=None,
                    in_=out_flat,
                    in_offset=IndirectOffsetOnAxis(ap=gg_sb[:, t : t + 1], axis=0),
                )
                add_dep_helper(gi.ins, barrier.ins, True, "gather after barrier")
                gather_insts.append(gi.ins)

            cosv = fix_p.tile([P, nt], f32, tag="cosv")
            nc.vector.tensor_scalar(
                out=cosv, in0=gat, scalar1=1.0 / S,
                scalar2=None, op0=ALU.mult,
            )
            cosc = fix_p.tile([P, nt], f32, tag="cosc")
            nc.vector.tensor_scalar(
                out=cosc, in0=cosv, scalar1=1.0 - EPS_CLIP,
                scalar2=-1.0 + EPS_CLIP, op0=ALU.min, op1=ALU.max,
            )
            ncsq = fix_p.tile([P, nt], f32, tag="ncsq")
            nc.vector.scalar_tensor_tensor(
                out=ncsq, in0=cosc, scalar=-1.0, in1=cosc,
                op0=ALU.mult, op1=ALU.mult,
            )
            s2 = fix_p.tile([P, nt], f32, tag="s2")
            nc.vector.tensor_scalar(
                out=s2, in0=ncsq, scalar1=1.0, scalar2=None, op0=ALU.add,
            )
            sn = fix_p.tile([P, nt], f32, tag="sn")
            nc.scalar.activation(
                out=sn, in_=s2, func=mybir.ActivationFunctionType.Sqrt,
            )
            # one Newton step: s <- 0.5*(s + s2/s) (ACT sqrt table is loose)
            rs = fix_p.tile([P, nt], f32, tag="rs")
            nc.vector.reciprocal(out=rs, in_=sn)
            t1 = fix_p.tile([P, nt], f32, tag="t1")
            nc.vector.tensor_tensor(out=t1, in0=s2, in1=rs, op=ALU.mult)
            t2 = fix_p.tile([P, nt], f32, tag="t2")
            nc.vector.tensor_tensor(out=t2, in0=sn, in1=t1, op=ALU.add)
            sref = fix_p.tile([P, nt], f32, tag="sref")
            nc.vector.tensor_scalar(
                out=sref, in0=t2, scalar1=0.5, scalar2=None, op0=ALU.mult,
            )
            t3 = fix_p.tile([P, nt], f32, tag="t3")
            nc.vector.tensor_scalar(
                out=t3, in0=sref, scalar1=S * math.sin(MARGIN),
                scalar2=None, op0=ALU.mult,
            )
            val = fix_p.tile([P, nt], f32, tag="val")
            nc.vector.scalar_tensor_tensor(
                out=val, in0=cosc, scalar=S * math.cos(MARGIN), in1=t3,
                op0=ALU.mult, op1=ALU.subtract,
            )
            barrier2 = nc.gpsimd.nop(nofuse=True, hint="scatter_barrier")
            for gins in gather_insts:
                add_dep_helper(barrier2.ins, gins, True, "scatters after gathers")
            for t in range(nt):
                si = nc.gpsimd.indirect_dma_start(
                    out=out_flat,
                    out_offset=IndirectOffsetOnAxis(ap=gs_sb[:, t : t + 1], axis=0),
                    in_=val[:, t : t + 1],
                    in_offset=None,
                    bounds_check=n * cs - 1,
                    oob_is_err=False,
                )
                add_dep_helper(si.ins, barrier2.ins, True, "scatter after barrier")
    return nc


def make_core_inputs(emb, weight_padded, labels, n, cs, core_id):
    """Host-side shard marshaling: weight slab + gather/scatter indices."""
    nt = n // P
    c0 = core_id * cs
    wshard = np.ascontiguousarray(weight_padded[c0 : c0 + cs])
    col = labels.astype(np.int64) - c0
    in_range = (col >= 0) & (col < cs)
    colc = np.clip(col, 0, cs - 1)
    flat = np.arange(n, dtype=np.int64) * cs + colc
    gidxg = flat.astype(np.int32).reshape(nt, P).T
    gidxs = np.where(in_range, flat, OOB_SENTINEL).astype(np.int32)
    gidxs = gidxs.reshape(nt, P).T
    return {
        "emb": emb,
        "weight": wshard,
        "gidxg": np.ascontiguousarray(gidxg),
        "gidxs": np.ascontiguousarray(gidxs),
    }


def kernel(emb, weight, labels, _trace=False, _trace_kwargs=None):
    global LAST_EXEC_NS, LAST_RESULTS
    emb = np.ascontiguousarray(np.asarray(emb, dtype=np.float32))
    weight = np.asarray(weight, dtype=np.float32)
    labels = np.asarray(labels)

    n, d = emb.shape
    c_full = weight.shape[0]
    assert (n, d) == (N_FULL, D_FULL) and c_full == C_FULL

    wpad = np.zeros((N_CORES * CS, d), dtype=np.float32)
    wpad[:c_full] = weight

    in_maps = [
        make_core_inputs(emb, wpad, labels, n, CS, i) for i in range(N_CORES)
    ]
    nc = build_arcface(n=n, d=d, cs=CS)
    nc.finalize()  # Bacc: split sync waits + allocate registers
    kwargs = {}
    if _trace:
        kwargs["trace"] = True
        if _trace_kwargs:
            kwargs.update(_trace_kwargs)
    res = run_bass_kernel_spmd(nc, in_maps, core_ids=list(range(N_CORES)), **kwargs)
    LAST_EXEC_NS = res.exec_time_ns
    LAST_RESULTS = res
    out = np.concatenate([res.results[i]["out"] for i in range(N_CORES)], axis=1)
    return np.ascontiguousarray(out[:, :c_full])
